# revision 1
# baseline (speedup 1.0000x reference)
"""Trainium2 Bass kernel: weighted sliding-window min (STL 'Always' robustness).

out[n, w] = min_k( input[n, 4*w + k] * And_weight[0, k] ),  k in [0, 16)

Strategy (8 NeuronCores, data-parallel over batch N=1024 -> 128 rows/core):
  - Host: cast input to bf16 and deinterleave each row into 4 phase planes
    P_j[b] = x[4b + j]; pre-tile along the block axis with a 3-block halo.
  - Device: the 16 products p_{o,j} = P_j * c[4o+j], split evenly between
    VectorE (tensor_scalar, bf16 4x mode; o in {0,1}) and ScalarE
    (ACTIVATE-with-scale; o in {2,3}), then a 4-level tensor_tensor min
    tree (bf16 2x_1p on DVE) with window shifts folded into access-pattern
    offsets (even shifts keep 4B alignment and hence the 2x packing mode).
    The first tree level is split by product origin so ScalarE's chain is
    only awaited by the pieces that consume it.
  - out[w] = min_o m_o[w+o] where m_o[b] = min_j P_j[b]*c[4o+j]; output is
    written bf16 (exact: a min picks one of the bf16 products) and upcast
    to float32 on the host.
"""

import numpy as np

# Problem geometry (hardcoded; harness calls kernel() with these shapes)
N, L = 1024, 8192
K, S = 16, 4
W = (L - K) // S + 1          # 2045 output windows per row
NCORES = 8
ROWS = N // NCORES            # 128 rows per core == SBUF partitions
B = L // S                    # 2048 blocks of 4 per row

# Column tiling over the block axis
import os as _os

NT = int(_os.environ.get("K_NT", "2"))   # number of column tiles
# BT=1024 (not ceil(W/2)=1023) measures ~1.5us faster; keep tiles pow2-sized.
_BT_DEFAULT = 1024 if NT == 2 else (W + NT - 1) // NT
BT = int(_os.environ.get("K_BT", "0")) or _BT_DEFAULT  # outputs per tile
TW = BT + 3                   # tile width in blocks (3-block halo)

# Which of the 16 products (o, j) run on ScalarE (the rest on VectorE),
# per column tile. ScalarE is ~2.5x slower per element but its muls run in
# the shadow of VectorE's min tree; later tiles have more shadow.
_ACT_N = [int(c) for c in _os.environ.get("K_ACT", "88")]
_ACT_ORDER = [(2, 0), (2, 1), (2, 2), (2, 3), (3, 0), (3, 1), (3, 2), (3, 3)]
ACT_MULS = [set(_ACT_ORDER[: _ACT_N[min(t, len(_ACT_N) - 1)]]) for t in range(NT)]

_COMPILED = {}


def _build_bass():
    import concourse.bacc as bacc
    import concourse.mybir as mybir
    from concourse.tile import TileContext

    BF16 = mybir.dt.bfloat16
    F32 = mybir.dt.float32
    MIN = mybir.AluOpType.min

    nc = bacc.Bacc(enable_partition_id=False)
    x = nc.dram_tensor("x", [ROWS, NT, 4, TW], BF16, kind="ExternalInput")
    w = nc.dram_tensor("w", [ROWS, 16], F32, kind="ExternalInput")
    out = nc.dram_tensor("out", [ROWS, W], BF16, kind="ExternalOutput")

    # slot(o, j): plane ordering that keeps every min-tree level a dense
    # step-1 access pattern:
    #   Q = [q0A q1A q0B q1B | q2A q3A q2B q3B]
    #   U = [uA vA uB vB],  R = [r0 r1]
    def slot(o, j):
        return 4 * (o // 2) + 2 * (j // 2) + (o % 2)

    with TileContext(nc) as tc:
        with (
            tc.tile_pool(name="wp", bufs=1) as wp,
            tc.tile_pool(name="xin", bufs=2) as xin,
            tc.tile_pool(name="pa", bufs=2) as pa,
            tc.tile_pool(name="pb", bufs=2) as pb,
            tc.tile_pool(name="qq", bufs=2) as qq,
            tc.tile_pool(name="uu", bufs=2) as uu,
            tc.tile_pool(name="rr", bufs=2) as rr,
            tc.tile_pool(name="oo", bufs=2) as oo,
        ):
            # Dummy first Activation so Bacc hoists the ACT table load to the
            # top of the Scalar queue (it otherwise lands behind the first
            # real ACTIVATE's DMA wait, delaying the whole ACT chain).
            dummy = wp.tile([ROWS, 1], F32)
            nc.scalar.memzero(dummy[:, :])
            w_sb = wp.tile([ROWS, 16], F32)
            nc.scalar.dma_start(out=w_sb[:, :], in_=w[:, :])

            # Hoist ALL input-plane DMA issues to the top, interleaved over
            # both HWDGE queues with tile 0 first on each: the rings serve
            # queues round-robin, so this guarantees tile 0's planes drain
            # before tile 1 takes bandwidth. Emitting the Scalar-queue issues
            # before the first ACTIVATE hides them in ACT's data wait.
            xts_all = []
            for t in range(NT):
                row = []
                for j in range(4):
                    if t == 0 and j == 0:
                        xa = xin.tile([ROWS, 512], BF16, tag="xt0a")
                        nc.sync.dma_start(out=xa[:, :], in_=x[:, t, j, 0:512])
                        xb = xin.tile([ROWS, TW - 512], BF16, tag="xt0b")
                        nc.sync.dma_start(out=xb[:, :], in_=x[:, t, j, 512:TW])
                        row.append((xa, xb))
                        continue
                    xtj = xin.tile([ROWS, TW], BF16, tag=f"xt{j}")
                    eng = nc.scalar if j % 2 == 1 else nc.sync
                    eng.dma_start(out=xtj[:, :], in_=x[:, t, j, :])
                    row.append(xtj)
                xts_all.append(row)

            for t in range(NT):
                wbase = BT * t
                wcnt = min(BT, W - wbase)
                xts = xts_all[t]

                def plane(j):
                    return xts[j][:, :]

                A = pa.tile([ROWS, 8, TW], BF16)
                Bb = pb.tile([ROWS, 8, TW], BF16)
                # 16 products p_{o,j} = P_j * c[4o+j].
                # j even -> A buf, j odd -> B buf (L1 pairs (j=0,1) and (j=2,3)).
                # ACT takes ACT_MULS of the 16 (its ACTIVATE is ~2.5x slower
                # per element but runs in the shadow of DVE's min tree).
                for j in range(4):
                    for o in range(4):
                        dst = A if (j % 2 == 0) else Bb
                        s = slot(o, j)
                        sc = w_sb[:, 4 * o + j : 4 * o + j + 1]
                        if t == 0 and j == 0:
                            xa, xb = xts[0]
                            if (o, j) in ACT_MULS[t]:
                                nc.scalar.mul(
                                    out=dst[:, s, 0:512], in_=xa[:, :], mul=sc
                                )
                                nc.scalar.mul(
                                    out=dst[:, s, 512:TW], in_=xb[:, :], mul=sc
                                )
                            else:
                                nc.vector.tensor_scalar_mul(
                                    out=dst[:, s, 0:512], in0=xa[:, :], scalar1=sc
                                )
                                nc.vector.tensor_scalar_mul(
                                    out=dst[:, s, 512:TW], in0=xb[:, :], scalar1=sc
                                )
                        elif (o, j) in ACT_MULS[t]:
                            nc.scalar.mul(out=dst[:, s, :], in_=plane(j), mul=sc)
                        else:
                            nc.vector.tensor_scalar_mul(
                                out=dst[:, s, :], in0=plane(j), scalar1=sc
                            )

                # L1: q = min(p_{o, j even}, p_{o, j odd})  (8 planes),
                # split by product origin: slots 0:4 depend only on DVE
                # products, slots 4:8 on the (later) ScalarE chain.
                Q = qq.tile([ROWS, 8, TW], BF16)
                nc.vector.tensor_tensor(
                    out=Q[:, 0:4, :], in0=A[:, 0:4, :], in1=Bb[:, 0:4, :], op=MIN
                )
                if t == NT - 1:
                    # last tile: sub-split so only the final piece waits for
                    # the very end of the ScalarE product chain
                    nc.vector.tensor_tensor(
                        out=Q[:, 4:6, :], in0=A[:, 4:6, :], in1=Bb[:, 4:6, :],
                        op=MIN,
                    )
                    nc.vector.tensor_tensor(
                        out=Q[:, 6:8, :], in0=A[:, 6:8, :], in1=Bb[:, 6:8, :],
                        op=MIN,
                    )
                else:
                    nc.vector.tensor_tensor(
                        out=Q[:, 4:8, :], in0=A[:, 4:8, :], in1=Bb[:, 4:8, :],
                        op=MIN,
                    )

                # L2: fold the o -> o+2 shift (+2 blocks, stays 4B-aligned)
                U = uu.tile([ROWS, 4, TW - 2], BF16)
                nc.vector.tensor_tensor(
                    out=U[:, :, :],
                    in0=Q[:, 0:4, 0 : TW - 2],
                    in1=Q[:, 4:8, 2:TW],
                    op=MIN,
                )

                # L3: A-half vs B-half
                R = rr.tile([ROWS, 2, TW - 2], BF16)
                nc.vector.tensor_tensor(
                    out=R[:, :, :], in0=U[:, 0:2, :], in1=U[:, 2:4, :], op=MIN
                )

                # L4: out[w] = min(r0[w], r1[w+1]). One op + ONE out-DMA per
                # tile: a trailing small DMA chunk measured ~2.7us WORSE (its
                # ~2us fixed completion latency queues behind the big chunk's
                # and gates the end barrier).
                ot = oo.tile([ROWS, BT], BF16)
                H = min(BT, wcnt)
                nc.vector.tensor_tensor(
                    out=ot[:, 0:H], in0=R[:, 0, 0:H], in1=R[:, 1, 1 : H + 1], op=MIN
                )
                nc.sync.dma_start(out=out[:, wbase : wbase + H], in_=ot[:, 0:H])
                if wcnt > H:
                    nc.vector.tensor_tensor(
                        out=ot[:, H:BT],
                        in0=R[:, 0, H:BT],
                        in1=R[:, 1, H + 1 : BT + 1],
                        op=MIN,
                    )
                    nc.sync.dma_start(
                        out=out[:, wbase + H : wbase + wcnt], in_=ot[:, H:wcnt]
                    )
    nc.finalize()
    return nc


def _host_prep(input_f32, And_weight):
    """Shard + relayout host-side. Returns in_maps for the 8 cores."""
    import ml_dtypes

    xb = np.asarray(input_f32, dtype=np.float32).astype(ml_dtypes.bfloat16)
    # [N, L] -> [N, B, 4] -> [N, 4, B] phase planes
    planes = np.ascontiguousarray(xb.reshape(N, B, S).transpose(0, 2, 1))
    # pad block axis B -> BT*NT + 3 so every tile has its halo
    padded = np.zeros((N, S, NT * BT + 3), dtype=ml_dtypes.bfloat16)
    padded[:, :, :B] = planes
    # [N, 4, padB] -> tiles [N, NT, 4, TW]
    xt = np.empty((N, NT, S, TW), dtype=ml_dtypes.bfloat16)
    for t in range(NT):
        xt[:, t] = padded[:, :, BT * t : BT * t + TW]

    wfull = np.broadcast_to(
        np.asarray(And_weight, dtype=np.float32).reshape(1, K), (ROWS, K)
    ).copy()

    in_maps = []
    for c in range(NCORES):
        in_maps.append(
            {
                "x": np.ascontiguousarray(xt[c * ROWS : (c + 1) * ROWS]),
                "w": wfull,
            }
        )
    return in_maps


def _get_nc():
    if "nc" not in _COMPILED:
        _COMPILED["nc"] = _build_bass()
    return _COMPILED["nc"]


def _run(in_maps, trace=False, **kw):
    from concourse.bass_utils import run_bass_kernel_spmd

    nc = _get_nc()
    res = run_bass_kernel_spmd(
        nc, in_maps, core_ids=list(range(NCORES)), trace=trace, **kw
    )
    return res


def kernel(input, And_weight):
    in_maps = _host_prep(input, And_weight)
    res = _run(in_maps, trace=False)
    out = np.concatenate([res.results[c]["out"] for c in range(NCORES)], axis=0)
    return out.astype(np.float32)



# revision 3
# speedup vs baseline: 1.1589x; 1.1589x over previous
"""Trainium2 Bass kernel: weighted sliding-window min (STL 'Always' robustness).

out[n, w] = min_k( input[n, 4*w + k] * And_weight[0, k] ),  k in [0, 16)

Strategy (8 NeuronCores, data-parallel over batch N=1024 -> 128 rows/core):
  - Host: cast input to bf16 and deinterleave each row into 4 phase planes
    P_j[b] = x[4b + j]; pre-tile along the block axis with a 4-block halo
    (TW even => every plane slot is 4B-aligned => DVE 4x/2x packing modes).
  - Device: the 16 products p_{o,j} = P_j * c[4o+j], split between VectorE
    (tensor_scalar, bf16 4x mode) and ScalarE (ACTIVATE-with-scale), then a
    4-level tensor_tensor min tree (bf16 2x_1p on DVE) with window shifts
    folded into access-pattern offsets.
  - out[w] = min_o m_o[w+o] where m_o[b] = min_j P_j[b]*c[4o+j]; output is
    written bf16 (exact: a min picks one of the bf16 products) and upcast
    to float32 on the host.

Queue layout: ALL input DMAs ride the Sync HWDGE ring (weight tile first, so
DVE can start as soon as the first half-plane lands); the Scalar ring is left
free so ScalarE spends its sequencer time on ACTIVATE products, not
descriptor generation. Final tile's output is stored in two chunks on the
two rings so the last chunk's ~2us completion latency overlaps the first's.
"""

import numpy as np

# Problem geometry (hardcoded; harness calls kernel() with these shapes)
N, L = 1024, 8192
K, S = 16, 4
W = (L - K) // S + 1          # 2045 output windows per row
NCORES = 8
ROWS = N // NCORES            # 128 rows per core == SBUF partitions
B = L // S                    # 2048 blocks of 4 per row

import os as _os

NT = int(_os.environ.get("K_NT", "2"))   # number of column tiles
_BT_DEFAULT = 1024 if NT == 2 else (W + NT - 1) // NT
BT = int(_os.environ.get("K_BT", "0")) or _BT_DEFAULT  # outputs per tile
TW = BT + 4                   # tile width in blocks (4-block halo, even)

# Which of the 16 products (o, j) run on ScalarE (the rest on VectorE),
# per column tile. ScalarE is ~2.9x slower per element but its muls run in
# the shadow of VectorE's min tree.
_ACT_N = [int(c, 16) for c in _os.environ.get("K_ACT", "88")]
_ACT_ORDER = [(2, 0), (2, 1), (2, 2), (2, 3), (3, 0), (3, 1), (3, 2), (3, 3),
              (1, 0), (1, 1), (1, 2), (1, 3)]
ACT_MULS = [set(_ACT_ORDER[: _ACT_N[min(t, len(_ACT_N) - 1)]]) for t in range(NT)]

# First-tile early-start: split plane j=0 into chunks of this many elements
# so the first product (and the DVE pipeline) starts as soon as possible.
CH0 = int(_os.environ.get("K_CH0", "512"))

_COMPILED = {}


def _build_bass():
    import concourse.bacc as bacc
    import concourse.mybir as mybir
    from concourse.tile import TileContext

    BF16 = mybir.dt.bfloat16
    F32 = mybir.dt.float32
    MIN = mybir.AluOpType.min

    nc = bacc.Bacc(enable_partition_id=False)
    x = nc.dram_tensor("x", [ROWS, NT, 4, TW], BF16, kind="ExternalInput")
    w = nc.dram_tensor("w", [ROWS, 16], F32, kind="ExternalInput")
    out = nc.dram_tensor("out", [ROWS, W], BF16, kind="ExternalOutput")

    # slot(o, j): plane ordering that keeps every min-tree level a dense
    # step-1 access pattern:
    #   Q = [q0A q1A q0B q1B | q2A q3A q2B q3B]
    #   U = [uA vA uB vB],  R = [r0 r1]
    def slot(o, j):
        return 4 * (o // 2) + 2 * (j // 2) + (o % 2)

    with TileContext(nc) as tc:
        with (
            tc.tile_pool(name="wp", bufs=1) as wp,
            tc.tile_pool(name="xin", bufs=2) as xin,
            tc.tile_pool(name="pa", bufs=2) as pa,
            tc.tile_pool(name="pb", bufs=2) as pb,
            tc.tile_pool(name="qq", bufs=2) as qq,
            tc.tile_pool(name="uu", bufs=2) as uu,
            tc.tile_pool(name="rr", bufs=2) as rr,
            tc.tile_pool(name="oo", bufs=2) as oo,
        ):
            # Weight DMA rides FIRST on the Sync ring: it gates every DVE
            # product, and the Scalar ring would delay it behind the ACT
            # table load.
            w_sb = wp.tile([ROWS, 16], F32)
            nc.sync.dma_start(out=w_sb[:, :], in_=w[:, :])

            # Dummy first Activation so Bacc hoists the ACT table load to the
            # top of the Scalar queue (it otherwise lands behind the first
            # real ACTIVATE's DMA wait, delaying the whole ACT chain).
            dummy = wp.tile([ROWS, 1], F32)
            nc.scalar.memzero(dummy[:, :])

            # ALL input-plane DMAs on the Sync ring, tile 0 first (FIFO per
            # ring => tile 0 gets full DMA bandwidth until it lands), with
            # plane j=0 of tile 0 chunked so DVE starts earliest.
            xts_all = []
            for t in range(NT):
                row = []
                for j in range(4):
                    if t == 0 and j == 0:
                        xa = xin.tile([ROWS, CH0], BF16, tag="xt0a")
                        nc.sync.dma_start(out=xa[:, :], in_=x[:, t, j, 0:CH0])
                        xb = xin.tile([ROWS, TW - CH0], BF16, tag="xt0b")
                        nc.sync.dma_start(out=xb[:, :], in_=x[:, t, j, CH0:TW])
                        row.append((xa, xb))
                        continue
                    xtj = xin.tile([ROWS, TW], BF16, tag=f"xt{j}")
                    nc.sync.dma_start(out=xtj[:, :], in_=x[:, t, j, :])
                    row.append(xtj)
                xts_all.append(row)

            for t in range(NT):
                wbase = BT * t
                wcnt = min(BT, W - wbase)
                xts = xts_all[t]

                def plane(j):
                    return xts[j][:, :]

                A = pa.tile([ROWS, 8, TW], BF16)
                Bb = pb.tile([ROWS, 8, TW], BF16)
                # 16 products p_{o,j} = P_j * c[4o+j].
                # j even -> A buf, j odd -> B buf (L1 pairs (j=0,1) and (j=2,3)).
                for j in range(4):
                    for o in range(4):
                        dst = A if (j % 2 == 0) else Bb
                        s = slot(o, j)
                        sc = w_sb[:, 4 * o + j : 4 * o + j + 1]
                        if t == 0 and j == 0:
                            xa, xb = xts[0]
                            if (o, j) in ACT_MULS[t]:
                                nc.scalar.mul(
                                    out=dst[:, s, 0:CH0], in_=xa[:, :], mul=sc
                                )
                                nc.scalar.mul(
                                    out=dst[:, s, CH0:TW], in_=xb[:, :], mul=sc
                                )
                            else:
                                nc.vector.tensor_scalar_mul(
                                    out=dst[:, s, 0:CH0], in0=xa[:, :], scalar1=sc
                                )
                                nc.vector.tensor_scalar_mul(
                                    out=dst[:, s, CH0:TW], in0=xb[:, :], scalar1=sc
                                )
                        elif (o, j) in ACT_MULS[t]:
                            nc.scalar.mul(out=dst[:, s, :], in_=plane(j), mul=sc)
                        else:
                            nc.vector.tensor_scalar_mul(
                                out=dst[:, s, :], in0=plane(j), scalar1=sc
                            )

                # L1: q = min(p_{o, j even}, p_{o, j odd})  (8 planes),
                # split by product origin: slots 0:4 depend only on DVE
                # products, slots 4:8 on the (later) ScalarE chain.
                Q = qq.tile([ROWS, 8, TW], BF16)
                nc.vector.tensor_tensor(
                    out=Q[:, 0:4, :], in0=A[:, 0:4, :], in1=Bb[:, 0:4, :], op=MIN
                )
                if t == NT - 1:
                    # last tile: sub-split so only the final piece waits for
                    # the very end of the ScalarE product chain
                    nc.vector.tensor_tensor(
                        out=Q[:, 4:6, :], in0=A[:, 4:6, :], in1=Bb[:, 4:6, :],
                        op=MIN,
                    )
                    nc.vector.tensor_tensor(
                        out=Q[:, 6:8, :], in0=A[:, 6:8, :], in1=Bb[:, 6:8, :],
                        op=MIN,
                    )
                else:
                    nc.vector.tensor_tensor(
                        out=Q[:, 4:8, :], in0=A[:, 4:8, :], in1=Bb[:, 4:8, :], op=MIN
                    )

                # L2: fold the o -> o+2 shift (+2 blocks, stays 4B-aligned)
                U = uu.tile([ROWS, 4, TW - 2], BF16)
                nc.vector.tensor_tensor(
                    out=U[:, :, :],
                    in0=Q[:, 0:4, 0 : TW - 2],
                    in1=Q[:, 4:8, 2:TW],
                    op=MIN,
                )

                # L3: A-half vs B-half
                R = rr.tile([ROWS, 2, TW - 2], BF16)
                nc.vector.tensor_tensor(
                    out=R[:, :, :], in0=U[:, 0:2, :], in1=U[:, 2:4, :], op=MIN
                )

                # L4: out[w] = min(r0[w], r1[w+1]).
                ot = oo.tile([ROWS, BT], BF16)
                H = min(BT, wcnt)
                if t == NT - 1:
                    # last tile: two chunks on the two HWDGE rings so the
                    # second store's completion latency overlaps the first's.
                    HH = H // 2
                    nc.vector.tensor_tensor(
                        out=ot[:, 0:HH], in0=R[:, 0, 0:HH], in1=R[:, 1, 1 : HH + 1],
                        op=MIN,
                    )
                    nc.sync.dma_start(out=out[:, wbase : wbase + HH], in_=ot[:, 0:HH])
                    nc.vector.tensor_tensor(
                        out=ot[:, HH:H], in0=R[:, 0, HH:H], in1=R[:, 1, HH + 1 : H + 1],
                        op=MIN,
                    )
                    nc.scalar.dma_start(
                        out=out[:, wbase + HH : wbase + H], in_=ot[:, HH:H]
                    )
                else:
                    nc.vector.tensor_tensor(
                        out=ot[:, 0:H], in0=R[:, 0, 0:H], in1=R[:, 1, 1 : H + 1],
                        op=MIN,
                    )
                    nc.sync.dma_start(out=out[:, wbase : wbase + H], in_=ot[:, 0:H])
    nc.finalize()
    return nc


def _host_prep(input_f32, And_weight):
    """Shard + relayout host-side. Returns in_maps for the 8 cores."""
    import ml_dtypes

    xb = np.asarray(input_f32, dtype=np.float32).astype(ml_dtypes.bfloat16)
    # [N, L] -> [N, B, 4] -> [N, 4, B] phase planes
    planes = np.ascontiguousarray(xb.reshape(N, B, S).transpose(0, 2, 1))
    # pad block axis B -> BT*NT + 4 so every tile has its halo
    padded = np.zeros((N, S, NT * BT + 4), dtype=ml_dtypes.bfloat16)
    padded[:, :, :B] = planes
    # [N, 4, padB] -> tiles [N, NT, 4, TW]
    xt = np.empty((N, NT, S, TW), dtype=ml_dtypes.bfloat16)
    for t in range(NT):
        xt[:, t] = padded[:, :, BT * t : BT * t + TW]

    wfull = np.broadcast_to(
        np.asarray(And_weight, dtype=np.float32).reshape(1, K), (ROWS, K)
    ).copy()

    in_maps = []
    for c in range(NCORES):
        in_maps.append(
            {
                "x": np.ascontiguousarray(xt[c * ROWS : (c + 1) * ROWS]),
                "w": wfull,
            }
        )
    return in_maps


def _get_nc():
    if "nc" not in _COMPILED:
        _COMPILED["nc"] = _build_bass()
    return _COMPILED["nc"]


def _run(in_maps, trace=False, **kw):
    from concourse.bass_utils import run_bass_kernel_spmd

    nc = _get_nc()
    res = run_bass_kernel_spmd(
        nc, in_maps, core_ids=list(range(NCORES)), trace=trace, **kw
    )
    return res


def kernel(input, And_weight):
    in_maps = _host_prep(input, And_weight)
    res = _run(in_maps, trace=False)
    out = np.concatenate([res.results[c]["out"] for c in range(NCORES)], axis=0)
    return out.astype(np.float32)
